# revision 14
# baseline (speedup 1.0000x reference)
"""LDEPool1d Trainium2 Bass kernel (v4).

Reference computation (B=16, T=800, D=256, K=64):
    delta = x[:,:,None,:] - mu[None,None,:,:]          # (B,T,K,D)
    dist  = sum(delta*delta, -1)                       # (B,T,K)
    llk   = -(prec*prec) * dist
    r     = softmax(llk, axis=-1)                      # over K
    r     = r / (sum(r, axis=1) + 1e-9)                # over T
    pool  = einsum('btk,btkd->bkd', r, delta)          # (B,K,D)
    out   = pool.reshape(B, K*D)

Kernel algebra (per batch b; prec is constant so -p2*||x_t||^2 cancels in
the softmax over k):
    G[t,k] = sum_d x[t,d] * (2*p2[k]*mu[k,d])          (mm1, fp32 [t,k])
    llk    = G + nb[k],  nb = -p2*||mu_k||^2
    The softmax bias uses C_t = rowmax(G) + nbmax - 40 so that
    e = exp(G - rowmax(G) + (nb - nbmax + 40)) stays in [~e-46, e+40]:
    no overflow, and components with tiny weights keep full relative
    precision (needed: dead components with S ~ 1e-9 amplify relative r
    errors by ~1/(S+1e-9) in the T-normalization).
    Z_t = sum_k e, r = e / Z_t  (the +40/nbmax shift cancels in r)
    S_k = sum_t r  (ones-column in mm2), M2 = r^T @ x  (mm2, fp32r)
    out = M2 * Sr - mu * (S*Sr),  Sr = 1/(S+1e-9)

mm1 computes llkT[k,t] = mu_s @ x^T with mu stationary and xT moving at
N=400 so the fp32r fast path (1 cyc/row) applies; mu_s is split into a
bf16 high part + fp32 residual (two accumulating matmuls) so only x's
~10-bit fp32r truncation remains (llk errors are exponentiated, and
dead components with S ~ 1e-9 amplify relative r errors by ~1/(S+1e-9),
so mu-side sloppiness would cost ~1e-2).  The transpose-back of llkT to
[t,k] runs in exact fp32.  Sharding: data-parallel over B across 8
cores (2 batches/core), mu/prec replicated.  No collectives.
"""

import sys

if "/opt/trn_rl_repo" not in sys.path:
    sys.path.insert(0, "/opt/trn_rl_repo")

import numpy as np

B, T, D, K = 16, 800, 256, 64
N_CORES = 8
B_LOC = B // N_CORES  # batches per core
EPS = 1e-9
ONE_F32_BITS = 0x3F800000

# T-chunks of <=128 rows (SBUF partition dim)
CHUNKS = [(t0, min(128, T - t0)) for t0 in range(0, T, 128)]
NCH = len(CHUNKS)  # 7: 6 x 128 + 32
TP = NCH * 128  # xT padded with zero t-columns so mm1 writes full rows


def _bc(ap, n):
    """Append a stride-0 inner dim of size n to an AP (broadcast)."""
    import concourse.bass as bass

    return bass.AP(ap.tensor, ap.offset, ap.ap + [[0, n]])


def _bc_mid(ap, n):
    """Insert a stride-0 dim of size n before the last dim of an AP."""
    import concourse.bass as bass

    return bass.AP(ap.tensor, ap.offset, ap.ap[:-1] + [[0, n]] + [ap.ap[-1]])


def _emit(tc, x_d, mu_d, prec_d, out_d):
    from concourse import mybir
    from concourse.masks import make_identity
    from contextlib import ExitStack

    f32 = mybir.dt.float32
    f32r = mybir.dt.float32r
    u32 = mybir.dt.uint32
    nc = tc.nc
    AF = mybir.ActivationFunctionType
    OP = mybir.AluOpType

    def r32(ap):
        return ap.bitcast(f32r)

    ctx = ExitStack()
    const = ctx.enter_context(tc.tile_pool(name="const", bufs=1))
    xta = ctx.enter_context(tc.tile_pool(name="ps_xta", bufs=2, space="PSUM"))
    xtb = ctx.enter_context(tc.tile_pool(name="ps_xtb", bufs=2, space="PSUM"))
    lkt = ctx.enter_context(tc.tile_pool(name="ps_lkt", bufs=1, space="PSUM"))
    lkp = ctx.enter_context(tc.tile_pool(name="ps_llk", bufs=1, space="PSUM"))
    ppp = ctx.enter_context(tc.tile_pool(name="ps_pp", bufs=1, space="PSUM"))

    # ---------------- constants / per-batch SBUF tiles ----------------
    identity = const.tile([128, 128], f32r)  # for the fp32r x transposes
    identity_f = const.tile([64, 64], f32)  # for the fp32 setup transposes
    mu_nat = const.tile([K, D], f32)
    prec_sb = const.tile([K, 1], f32)
    p2 = const.tile([K, 1], f32)
    p22 = const.tile([K, 1], f32)
    mu_s = const.tile([K, D], f32)
    sq = const.tile([K, D], f32)
    musq = const.tile([K, 1], f32)
    nb = const.tile([K, 1], f32)
    muTh = const.tile([128, 2, K], f32r)  # fp32r-rounded high part
    muTl = const.tile([128, 2, K], f32r)  # residual
    nbs_row = const.tile([1, K], f32)  # nb - nbmax + 40, as a row
    nbmax = const.tile([1, 1], f32)
    nbsh = const.tile([1, 1], f32)
    ones_col = const.tile([1, 128], f32)
    nbs_rep = const.tile([128, K], f32)  # nbs_row replicated to all parts
    dum = const.tile([1, 1], f32)

    x_sb, xT_sb, lkT_sb, epre, e_sb, r_sb = [], [], [], [], [], []
    nm, nmnb, z, zinv, se, sr, c1, t1, t2, po = ([] for _ in range(10))
    for b in range(B_LOC):
        x_sb.append(const.tile([128, NCH, D + 2], f32r, tag=f"x{b}", name=f"x{b}"))
        xT_sb.append(const.tile([128, 2, T], f32r, tag=f"xT{b}", name=f"xT{b}"))
        lkT_sb.append(const.tile([64, TP], f32, tag=f"lkT{b}", name=f"lkT{b}"))
        epre.append(const.tile([128, NCH, K], f32, tag=f"ep{b}", name=f"ep{b}"))
        e_sb.append(const.tile([128, NCH, K], f32, tag=f"e{b}", name=f"e{b}"))
        r_sb.append(const.tile([128, NCH, K], f32r, tag=f"r{b}", name=f"r{b}"))
        nm.append(const.tile([128, NCH], f32, tag=f"nm{b}", name=f"nm{b}"))
        nmnb.append(const.tile([128, NCH, K], f32, tag=f"nn{b}", name=f"nn{b}"))
        z.append(const.tile([128, NCH], f32, tag=f"z{b}", name=f"z{b}"))
        zinv.append(const.tile([128, NCH], f32, tag=f"zi{b}", name=f"zi{b}"))
        se.append(const.tile([K, 1], f32, tag=f"se{b}", name=f"se{b}"))
        sr.append(const.tile([K, 1], f32, tag=f"sr{b}", name=f"sr{b}"))
        c1.append(const.tile([K, 1], f32, tag=f"c1{b}", name=f"c1{b}"))
        t1.append(const.tile([K, D], f32, tag=f"t1{b}", name=f"t1{b}"))
        t2.append(const.tile([K, D], f32, tag=f"t2{b}", name=f"t2{b}"))
        po.append(const.tile([K, D], f32, tag=f"po{b}", name=f"po{b}"))

    # ---------------- setup ----------------
    # mu/prec ride the ACT engine's DMA queue (tiny, lands early, and
    # does not delay the x transfers on the sync queue); then prefetch
    # the exp table set on ACT before anything else needs ACT.
    nc.scalar.dma_start(out=mu_nat, in_=mu_d)
    nc.scalar.dma_start(out=prec_sb, in_=prec_d)
    nc.gpsimd.memset(dum, 0.0)
    nc.scalar.activation(dum, dum, AF.Exp)

    # mu/prec first so the mu math can start immediately; x in 3 chunks
    # per batch (big DMAs, but the first lands early enough for trx).
    def dma_x(b, part):
        if part == 0:
            nc.sync.dma_start(
                out=x_sb[b][:, 0:3, 0:D],
                in_=x_d[b, 0:384, :].rearrange("(c p) d -> p c d", p=128).bitcast(f32r),
            )
        elif part == 1:
            nc.sync.dma_start(
                out=x_sb[b][:, 3:6, 0:D],
                in_=x_d[b, 384:768, :].rearrange("(c p) d -> p c d", p=128).bitcast(f32r),
            )
        else:
            nc.sync.dma_start(
                out=x_sb[b][0:32, 6, 0:D], in_=x_d[b, 768:800, :].bitcast(f32r)
            )

    for b in range(B_LOC):
        for part in range(3):
            dma_x(b, part)

    # Identities. The f32r one is built in place: uint32 memset for the
    # zeros (no f32r memset encoding exists) + affine_select for the diag.
    nc.gpsimd.memset(identity.bitcast(u32), 0)
    make_identity(nc, identity, nomemset=True)
    make_identity(nc, identity_f)
    nc.gpsimd.memset(ones_col, 1.0)
    for b in range(B_LOC):
        nc.gpsimd.memset(lkT_sb[b][:, T:TP], 0.0)
        # ones col for S_k (+ zero col so the fp32r moving dim is even)
        nc.gpsimd.memset(x_sb[b][:, :, D : D + 1].bitcast(u32), ONE_F32_BITS)
        nc.gpsimd.memset(x_sb[b][:, :, D + 1 : D + 2].bitcast(u32), 0)

    # mu math: p2 = prec^2; mu_s = 2*p2*mu; nb = -p2*||mu||^2
    nc.vector.tensor_mul(p2, prec_sb, prec_sb)
    nc.vector.tensor_scalar_mul(p22, p2, 2.0)
    nc.scalar.activation(mu_s, mu_nat, AF.Copy, scale=p22)
    nc.vector.tensor_mul(sq, mu_nat, mu_nat)
    nc.vector.tensor_reduce(
        out=musq, in_=sq, axis=mybir.AxisListType.X, op=OP.add
    )
    nc.vector.tensor_mul(nb, p2, musq)
    nc.vector.tensor_scalar_mul(nb, nb, -1.0)

    def setup_mu_transposes():
        # Transpose mu_s halves and nb via PE (fp32), staged through one
        # slot of the llk psum pool; split mu into an fp32r-rounded high
        # part + residual straight from PSUM.
        wrp = lkp.tile([128, NCH, K], f32, tag="llk")
        nc.tensor.transpose(wrp[:, 0, :], mu_s[:, 0:128], identity_f)
        nc.tensor.transpose(wrp[:, 1, :], mu_s[:, 128:256], identity_f)
        nc.tensor.transpose(wrp[0:1, 4, :], nb[:, 0:1], identity_f)
        nc.scalar.copy(muTh, wrp[:, 0:2, :])  # rounds to f32r
        nc.vector.tensor_tensor(
            out=muTl, in0=wrp[:, 0:2, :], in1=muTh, op=OP.subtract
        )
        # nbs_row = nb - nbmax + 40 (exponent offset; cancels in r)
        nc.vector.tensor_reduce(
            out=nbmax, in_=wrp[0:1, 4, :], axis=mybir.AxisListType.X,
            op=OP.max, negate=True,
        )
        nc.vector.tensor_scalar_add(nbsh, nbmax, 40.0)
        nc.vector.tensor_scalar(
            out=nbs_row, in0=wrp[0:1, 4, :], scalar1=nbsh, scalar2=None,
            op0=OP.add,
        )
        # Replicate nbs_row across all 128 partitions (rank-1 matmul).
        nc.tensor.matmul(wrp[:, 5, :], lhsT=ones_col, rhs=nbs_row)
        nc.scalar.copy(nbs_rep, wrp[:, 5, :])

    # ---------------- per-batch pipeline stages ----------------
    state = {}

    def trx(b):
        """Transpose x into xT (PSUM); fp32r (1.5 cyc/row)."""
        st = state.setdefault(b, {})
        st["xtps"] = []
        for h in range(2):
            pa = xta.tile([128, 512], f32, tag="xta")
            pb = xtb.tile([128, 288], f32, tag="xtb")
            for c, (t0, tcn) in enumerate(CHUNKS):
                dst = (
                    pa[:, t0 : t0 + tcn]
                    if t0 + tcn <= 512
                    else pb[:, t0 - 512 : t0 - 512 + tcn]
                )
                nc.tensor.matmul(
                    r32(dst),
                    lhsT=x_sb[b][0:tcn, c, h * 128 : (h + 1) * 128],
                    rhs=identity[0:tcn, 0:tcn],
                    is_transpose=True,
                )
            st["xtps"].append((pa, pb))

    def copy_xT(b, h):
        pa, pb = state[b]["xtps"][h]
        eng = nc.scalar if h == 0 else nc.vector
        if h == 0:
            eng.copy(xT_sb[b][:, h, 0:512], pa)
            eng.copy(xT_sb[b][:, h, 512:T], pb)
        else:
            eng.tensor_copy(xT_sb[b][:, h, 0:512], pa)
            eng.tensor_copy(xT_sb[b][:, h, 512:T], pb)

    def mm1(b):
        """llkT[k,t] = sum_d mu_s[k,d] x[t,d]; fp32r, mu stationary N=400."""
        pt = lkt.tile([64, 2, 512], f32, tag="lkT")
        for tc in range(2):
            for j in range(4):  # (h0,hi) (h0,lo) (h1,hi) (h1,lo)
                h, lo = j // 2, j % 2
                nc.tensor.matmul(
                    pt[:, tc, 0:400],
                    lhsT=(muTl if lo else muTh)[:, h, :],
                    rhs=xT_sb[b][:, h, tc * 400 : (tc + 1) * 400],
                    start=(j == 0),
                    stop=(j == 3),
                )
        state[b]["lkTps"] = pt

    def copy_lkT(b):
        pt = state[b]["lkTps"]
        nc.scalar.copy(lkT_sb[b][:, 0:400], pt[:, 0, 0:400])
        nc.scalar.copy(lkT_sb[b][:, 400:T], pt[:, 1, 0:400])

    def trllk(b):
        """llk[t,k] = llkT^T via exact fp32 PE transposes."""
        pl = lkp.tile([128, NCH, K], f32, tag="llk")
        for c in range(NCH):
            nc.tensor.transpose(
                pl[:, c, :],
                lkT_sb[b][:, c * 128 : (c + 1) * 128],
                identity_f,
            )
        state[b]["llkps"] = pl

    def softmax(b):
        pl = state[b]["llkps"]
        nc.vector.tensor_reduce(
            out=nm[b], in_=pl, axis=mybir.AxisListType.X, op=OP.max, negate=True
        )
        # nmnb[p,c,k] = -rowmax(G) + nb[k] - nbmax + 40
        nc.gpsimd.tensor_tensor(
            out=nmnb[b], in0=_bc(nm[b], K), in1=_bc_mid(nbs_rep, NCH), op=OP.add
        )
        nc.vector.tensor_tensor(out=epre[b], in0=pl, in1=nmnb[b], op=OP.add)
        nc.scalar.activation(e_sb[b], epre[b], AF.Exp)
        nc.vector.tensor_reduce(
            out=z[b], in_=e_sb[b], axis=mybir.AxisListType.X, op=OP.add
        )
        nc.vector.reciprocal(zinv[b], z[b])
        nc.gpsimd.tensor_tensor(
            out=r_sb[b], in0=e_sb[b], in1=_bc(zinv[b], K), op=OP.mult
        )

    def mm2(b):
        pp = ppp.tile([K, D + 2], f32, tag="pp")
        for c, (t0, tcn) in enumerate(CHUNKS):
            nc.tensor.matmul(
                pp,
                lhsT=r_sb[b][0:tcn, c, :],
                rhs=x_sb[b][0:tcn, c, 0 : D + 2],
                start=(c == 0),
                stop=(c == NCH - 1),
            )
        state[b]["pp"] = pp

    def epilogue(b):
        pp = state[b]["pp"]
        nc.vector.tensor_scalar_add(se[b], pp[:, D : D + 1], EPS)
        nc.vector.reciprocal(sr[b], se[b])
        nc.vector.tensor_mul(c1[b], pp[:, D : D + 1], sr[b])
        nc.scalar.activation(t2[b], pp[:, 0:D], AF.Copy, scale=sr[b])
        nc.scalar.activation(t1[b], mu_nat, AF.Copy, scale=c1[b])
        nc.gpsimd.tensor_tensor(out=po[b], in0=t2[b], in1=t1[b], op=OP.subtract)
        nc.sync.dma_start(
            out=out_d[b, :].rearrange("(k d) -> k d", k=K), in_=po[b]
        )

    # Interleave the two batches to keep PE busy during softmax/copies.
    trx(0)
    setup_mu_transposes()
    copy_xT(0, 0)
    copy_xT(0, 1)
    trx(1)
    mm1(0)
    copy_lkT(0)
    trllk(0)
    copy_xT(1, 0)
    copy_xT(1, 1)
    softmax(0)
    mm1(1)
    copy_lkT(1)
    trllk(1)
    mm2(0)
    softmax(1)
    epilogue(0)
    mm2(1)
    epilogue(1)
    ctx.close()


_NC = None


def _get_nc():
    global _NC
    if _NC is None:
        import concourse.bacc as bacc
        import concourse.tile as tile
        from concourse import mybir

        f32 = mybir.dt.float32
        nc = bacc.Bacc(
            "TRN2", target_bir_lowering=False, debug=False, num_devices=N_CORES
        )
        x_d = nc.dram_tensor("x", [B_LOC, T, D], f32, kind="ExternalInput").ap()
        mu_d = nc.dram_tensor("mu", [K, D], f32, kind="ExternalInput").ap()
        prec_d = nc.dram_tensor("prec", [K], f32, kind="ExternalInput").ap()
        out_d = nc.dram_tensor(
            "out", [B_LOC, K * D], f32, kind="ExternalOutput"
        ).ap()
        with tile.TileContext(nc) as tc:
            _emit(tc, x_d, mu_d, prec_d, out_d)
        nc.compile()
        _NC = nc
    return _NC


def kernel(x, mu, prec, **_ignored):
    from concourse.bass_utils import run_bass_kernel_spmd

    x = np.ascontiguousarray(np.asarray(x, dtype=np.float32))
    mu = np.ascontiguousarray(np.asarray(mu, dtype=np.float32))
    prec = np.ascontiguousarray(np.asarray(prec, dtype=np.float32))
    nc = _get_nc()
    in_maps = [
        {"x": x[c * B_LOC : (c + 1) * B_LOC], "mu": mu, "prec": prec}
        for c in range(N_CORES)
    ]
    res = run_bass_kernel_spmd(nc, in_maps, list(range(N_CORES)))
    return np.concatenate(
        [res.results[c]["out"] for c in range(N_CORES)], axis=0
    ).astype(np.float32)


# revision 15
# speedup vs baseline: 1.1545x; 1.1545x over previous
"""LDEPool1d Trainium2 Bass kernel (v4).

Reference computation (B=16, T=800, D=256, K=64):
    delta = x[:,:,None,:] - mu[None,None,:,:]          # (B,T,K,D)
    dist  = sum(delta*delta, -1)                       # (B,T,K)
    llk   = -(prec*prec) * dist
    r     = softmax(llk, axis=-1)                      # over K
    r     = r / (sum(r, axis=1) + 1e-9)                # over T
    pool  = einsum('btk,btkd->bkd', r, delta)          # (B,K,D)
    out   = pool.reshape(B, K*D)

Kernel algebra (per batch b; prec is constant so -p2*||x_t||^2 cancels in
the softmax over k):
    G[t,k] = sum_d x[t,d] * (2*p2[k]*mu[k,d])          (mm1, fp32 [t,k])
    llk    = G + nb[k],  nb = -p2*||mu_k||^2
    The softmax bias uses C_t = rowmax(G) + nbmax - 40 so that
    e = exp(G - rowmax(G) + (nb - nbmax + 40)) stays in [~e-46, e+40]:
    no overflow, and components with tiny weights keep full relative
    precision (needed: dead components with S ~ 1e-9 amplify relative r
    errors by ~1/(S+1e-9) in the T-normalization).
    Z_t = sum_k e, r = e / Z_t  (the +40/nbmax shift cancels in r)
    S_k = sum_t r  (ones-column in mm2), M2 = r^T @ x  (mm2, fp32r)
    out = M2 * Sr - mu * (S*Sr),  Sr = 1/(S+1e-9)

mm1 computes llkT[k,t] = mu_s @ x^T with mu stationary and xT moving at
N=400 so the fp32r fast path (1 cyc/row) applies; mu_s is split into a
bf16 high part + fp32 residual (two accumulating matmuls) so only x's
~10-bit fp32r truncation remains (llk errors are exponentiated, and
dead components with S ~ 1e-9 amplify relative r errors by ~1/(S+1e-9),
so mu-side sloppiness would cost ~1e-2).  The transpose-back of llkT to
[t,k] runs in exact fp32.  Sharding: data-parallel over B across 8
cores (2 batches/core), mu/prec replicated.  No collectives.
"""

import sys

if "/opt/trn_rl_repo" not in sys.path:
    sys.path.insert(0, "/opt/trn_rl_repo")

import numpy as np

B, T, D, K = 16, 800, 256, 64
N_CORES = 8
B_LOC = B // N_CORES  # batches per core
EPS = 1e-9
ONE_F32_BITS = 0x3F800000

# T-chunks of <=128 rows (SBUF partition dim)
CHUNKS = [(t0, min(128, T - t0)) for t0 in range(0, T, 128)]
NCH = len(CHUNKS)  # 7: 6 x 128 + 32
TP = NCH * 128  # xT padded with zero t-columns so mm1 writes full rows


def _bc(ap, n):
    """Append a stride-0 inner dim of size n to an AP (broadcast)."""
    import concourse.bass as bass

    return bass.AP(ap.tensor, ap.offset, ap.ap + [[0, n]])


def _bc_mid(ap, n):
    """Insert a stride-0 dim of size n before the last dim of an AP."""
    import concourse.bass as bass

    return bass.AP(ap.tensor, ap.offset, ap.ap[:-1] + [[0, n]] + [ap.ap[-1]])


def _emit(tc, x_d, mu_d, prec_d, out_d):
    from concourse import mybir
    from concourse.masks import make_identity
    from contextlib import ExitStack

    f32 = mybir.dt.float32
    f32r = mybir.dt.float32r
    u32 = mybir.dt.uint32
    nc = tc.nc
    AF = mybir.ActivationFunctionType
    OP = mybir.AluOpType

    def r32(ap):
        return ap.bitcast(f32r)

    ctx = ExitStack()
    const = ctx.enter_context(tc.tile_pool(name="const", bufs=1))
    xta = ctx.enter_context(tc.tile_pool(name="ps_xta", bufs=2, space="PSUM"))
    xtb = ctx.enter_context(tc.tile_pool(name="ps_xtb", bufs=2, space="PSUM"))
    lkt = ctx.enter_context(tc.tile_pool(name="ps_lkt", bufs=1, space="PSUM"))
    lkp = ctx.enter_context(tc.tile_pool(name="ps_llk", bufs=1, space="PSUM"))
    ppp = ctx.enter_context(tc.tile_pool(name="ps_pp", bufs=1, space="PSUM"))

    # ---------------- constants / per-batch SBUF tiles ----------------
    identity = const.tile([128, 128], f32r)  # for the fp32r x transposes
    identity_f = const.tile([64, 64], f32)  # for the fp32 setup transposes
    mu_nat = const.tile([K, D], f32)
    prec_sb = const.tile([K, 1], f32)
    p2 = const.tile([K, 1], f32)
    p22 = const.tile([K, 1], f32)
    mu_s = const.tile([K, D], f32)
    sq = const.tile([K, D], f32)
    musq = const.tile([K, 1], f32)
    nb = const.tile([K, 1], f32)
    muTh = const.tile([128, 2, K], f32r)  # fp32r-rounded high part
    muTl = const.tile([128, 2, K], f32r)  # residual
    nbs_row = const.tile([1, K], f32)  # nb - nbmax + 40, as a row
    nbmax = const.tile([1, 1], f32)
    nbsh = const.tile([1, 1], f32)
    ones_col = const.tile([1, 128], f32)
    nbs_rep = const.tile([128, K], f32)  # nbs_row replicated to all parts
    dum = const.tile([1, 1], f32)

    x_sb, xT_sb, lkT_sb, epre, e_sb, r_sb = [], [], [], [], [], []
    nm, nmnb, z, zinv, se, sr, c1, t1, t2, po = ([] for _ in range(10))
    for b in range(B_LOC):
        x_sb.append(const.tile([128, NCH, D + 2], f32r, tag=f"x{b}", name=f"x{b}"))
        xT_sb.append(const.tile([128, 2, T], f32r, tag=f"xT{b}", name=f"xT{b}"))
        lkT_sb.append(const.tile([64, TP], f32, tag=f"lkT{b}", name=f"lkT{b}"))
        epre.append(const.tile([128, NCH, K], f32, tag=f"ep{b}", name=f"ep{b}"))
        e_sb.append(const.tile([128, NCH, K], f32, tag=f"e{b}", name=f"e{b}"))
        r_sb.append(const.tile([128, NCH, K], f32r, tag=f"r{b}", name=f"r{b}"))
        nm.append(const.tile([128, NCH], f32, tag=f"nm{b}", name=f"nm{b}"))
        nmnb.append(const.tile([128, NCH, K], f32, tag=f"nn{b}", name=f"nn{b}"))
        z.append(const.tile([128, NCH], f32, tag=f"z{b}", name=f"z{b}"))
        zinv.append(const.tile([128, NCH], f32, tag=f"zi{b}", name=f"zi{b}"))
        se.append(const.tile([K, 1], f32, tag=f"se{b}", name=f"se{b}"))
        sr.append(const.tile([K, 1], f32, tag=f"sr{b}", name=f"sr{b}"))
        c1.append(const.tile([K, 1], f32, tag=f"c1{b}", name=f"c1{b}"))
        t1.append(const.tile([K, D], f32, tag=f"t1{b}", name=f"t1{b}"))
        t2.append(const.tile([K, D], f32, tag=f"t2{b}", name=f"t2{b}"))
        po.append(const.tile([K, D], f32, tag=f"po{b}", name=f"po{b}"))

    # ---------------- setup ----------------
    # Prefetch the exp table set on ACT before anything else needs ACT.
    nc.gpsimd.memset(dum, 0.0)
    nc.scalar.activation(dum, dum, AF.Exp)

    # mu/prec first so the mu math can start immediately; x in 3 chunks
    # per batch (big DMAs, but the first lands early enough for trx).
    def dma_x(b, part):
        if part == 0:
            nc.sync.dma_start(
                out=x_sb[b][:, 0:3, 0:D],
                in_=x_d[b, 0:384, :].rearrange("(c p) d -> p c d", p=128).bitcast(f32r),
            )
        elif part == 1:
            nc.sync.dma_start(
                out=x_sb[b][:, 3:6, 0:D],
                in_=x_d[b, 384:768, :].rearrange("(c p) d -> p c d", p=128).bitcast(f32r),
            )
        else:
            nc.sync.dma_start(
                out=x_sb[b][0:32, 6, 0:D], in_=x_d[b, 768:800, :].bitcast(f32r)
            )

    dma_x(0, 0)
    nc.sync.dma_start(out=mu_nat, in_=mu_d)
    nc.sync.dma_start(out=prec_sb, in_=prec_d)
    dma_x(0, 1)
    dma_x(0, 2)
    dma_x(1, 0)
    dma_x(1, 1)
    dma_x(1, 2)

    # Identities. The f32r one is built in place: uint32 memset for the
    # zeros (no f32r memset encoding exists) + affine_select for the diag.
    nc.gpsimd.memset(identity.bitcast(u32), 0)
    make_identity(nc, identity, nomemset=True)
    make_identity(nc, identity_f)
    nc.gpsimd.memset(ones_col, 1.0)
    for b in range(B_LOC):
        nc.gpsimd.memset(lkT_sb[b][:, T:TP], 0.0)
        # ones col for S_k (+ zero col so the fp32r moving dim is even)
        nc.gpsimd.memset(x_sb[b][:, :, D : D + 1].bitcast(u32), ONE_F32_BITS)
        nc.gpsimd.memset(x_sb[b][:, :, D + 1 : D + 2].bitcast(u32), 0)

    # mu math: p2 = prec^2; mu_s = 2*p2*mu; nb = -p2*||mu||^2
    nc.vector.tensor_mul(p2, prec_sb, prec_sb)
    nc.vector.tensor_scalar_mul(p22, p2, 2.0)
    nc.scalar.activation(mu_s, mu_nat, AF.Copy, scale=p22)
    nc.vector.tensor_mul(sq, mu_nat, mu_nat)
    nc.vector.tensor_reduce(
        out=musq, in_=sq, axis=mybir.AxisListType.X, op=OP.add
    )
    nc.vector.tensor_mul(nb, p2, musq)
    nc.vector.tensor_scalar_mul(nb, nb, -1.0)

    def setup_mu_transposes():
        # Transpose mu_s halves and nb via PE (fp32), staged through one
        # slot of the llk psum pool; split mu into an fp32r-rounded high
        # part + residual straight from PSUM.
        wrp = lkp.tile([128, NCH, K], f32, tag="llk")
        nc.tensor.transpose(wrp[:, 0, :], mu_s[:, 0:128], identity_f)
        nc.tensor.transpose(wrp[:, 1, :], mu_s[:, 128:256], identity_f)
        nc.tensor.transpose(wrp[0:1, 4, :], nb[:, 0:1], identity_f)
        nc.scalar.copy(muTh, wrp[:, 0:2, :])  # rounds to f32r
        nc.vector.tensor_tensor(
            out=muTl, in0=wrp[:, 0:2, :], in1=muTh, op=OP.subtract
        )
        # nbs_row = nb - nbmax + 40 (exponent offset; cancels in r)
        nc.vector.tensor_reduce(
            out=nbmax, in_=wrp[0:1, 4, :], axis=mybir.AxisListType.X,
            op=OP.max, negate=True,
        )
        nc.vector.tensor_scalar_add(nbsh, nbmax, 40.0)
        nc.vector.tensor_scalar(
            out=nbs_row, in0=wrp[0:1, 4, :], scalar1=nbsh, scalar2=None,
            op0=OP.add,
        )
        # Replicate nbs_row across all 128 partitions (rank-1 matmul).
        nc.tensor.matmul(wrp[:, 5, :], lhsT=ones_col, rhs=nbs_row)
        nc.scalar.copy(nbs_rep, wrp[:, 5, :])

    # ---------------- per-batch pipeline stages ----------------
    state = {}

    def trx(b):
        """Transpose x into xT (PSUM); fp32r (1.5 cyc/row)."""
        st = state.setdefault(b, {})
        st["xtps"] = []
        for h in range(2):
            pa = xta.tile([128, 512], f32, tag="xta")
            pb = xtb.tile([128, 288], f32, tag="xtb")
            for c, (t0, tcn) in enumerate(CHUNKS):
                dst = (
                    pa[:, t0 : t0 + tcn]
                    if t0 + tcn <= 512
                    else pb[:, t0 - 512 : t0 - 512 + tcn]
                )
                nc.tensor.matmul(
                    r32(dst),
                    lhsT=x_sb[b][0:tcn, c, h * 128 : (h + 1) * 128],
                    rhs=identity[0:tcn, 0:tcn],
                    is_transpose=True,
                )
            st["xtps"].append((pa, pb))

    def copy_xT(b, h):
        pa, pb = state[b]["xtps"][h]
        eng = nc.scalar if h == 0 else nc.vector
        if h == 0:
            eng.copy(xT_sb[b][:, h, 0:512], pa)
            eng.copy(xT_sb[b][:, h, 512:T], pb)
        else:
            eng.tensor_copy(xT_sb[b][:, h, 0:512], pa)
            eng.tensor_copy(xT_sb[b][:, h, 512:T], pb)

    def mm1(b):
        """llkT[k,t] = sum_d mu_s[k,d] x[t,d]; fp32r, mu stationary N=400."""
        pt = lkt.tile([64, 2, 512], f32, tag="lkT")
        for tc in range(2):
            for j in range(4):  # (h0,hi) (h0,lo) (h1,hi) (h1,lo)
                h, lo = j // 2, j % 2
                nc.tensor.matmul(
                    pt[:, tc, 0:400],
                    lhsT=(muTl if lo else muTh)[:, h, :],
                    rhs=xT_sb[b][:, h, tc * 400 : (tc + 1) * 400],
                    start=(j == 0),
                    stop=(j == 3),
                )
        state[b]["lkTps"] = pt

    def copy_lkT(b):
        pt = state[b]["lkTps"]
        nc.scalar.copy(lkT_sb[b][:, 0:400], pt[:, 0, 0:400])
        nc.scalar.copy(lkT_sb[b][:, 400:T], pt[:, 1, 0:400])

    def trllk(b):
        """llk[t,k] = llkT^T via exact fp32 PE transposes."""
        pl = lkp.tile([128, NCH, K], f32, tag="llk")
        for c in range(NCH):
            nc.tensor.transpose(
                pl[:, c, :],
                lkT_sb[b][:, c * 128 : (c + 1) * 128],
                identity_f,
            )
        state[b]["llkps"] = pl

    def softmax(b):
        pl = state[b]["llkps"]
        nc.vector.tensor_reduce(
            out=nm[b], in_=pl, axis=mybir.AxisListType.X, op=OP.max, negate=True
        )
        # nmnb[p,c,k] = -rowmax(G) + nb[k] - nbmax + 40
        nc.gpsimd.tensor_tensor(
            out=nmnb[b], in0=_bc(nm[b], K), in1=_bc_mid(nbs_rep, NCH), op=OP.add
        )
        nc.vector.tensor_tensor(out=epre[b], in0=pl, in1=nmnb[b], op=OP.add)
        nc.scalar.activation(e_sb[b], epre[b], AF.Exp)
        nc.vector.tensor_reduce(
            out=z[b], in_=e_sb[b], axis=mybir.AxisListType.X, op=OP.add
        )
        nc.vector.reciprocal(zinv[b], z[b])
        nc.gpsimd.tensor_tensor(
            out=r_sb[b], in0=e_sb[b], in1=_bc(zinv[b], K), op=OP.mult
        )

    def mm2(b):
        pp = ppp.tile([K, D + 2], f32, tag="pp")
        for c, (t0, tcn) in enumerate(CHUNKS):
            nc.tensor.matmul(
                pp,
                lhsT=r_sb[b][0:tcn, c, :],
                rhs=x_sb[b][0:tcn, c, 0 : D + 2],
                start=(c == 0),
                stop=(c == NCH - 1),
            )
        state[b]["pp"] = pp

    def epilogue(b):
        pp = state[b]["pp"]
        nc.vector.tensor_scalar_add(se[b], pp[:, D : D + 1], EPS)
        nc.vector.reciprocal(sr[b], se[b])
        nc.vector.tensor_mul(c1[b], pp[:, D : D + 1], sr[b])
        nc.scalar.activation(t2[b], pp[:, 0:D], AF.Copy, scale=sr[b])
        nc.scalar.activation(t1[b], mu_nat, AF.Copy, scale=c1[b])
        nc.gpsimd.tensor_tensor(out=po[b], in0=t2[b], in1=t1[b], op=OP.subtract)
        nc.sync.dma_start(
            out=out_d[b, :].rearrange("(k d) -> k d", k=K), in_=po[b]
        )

    # Interleave the two batches to keep PE busy during softmax/copies.
    setup_mu_transposes()
    trx(0)
    copy_xT(0, 0)
    copy_xT(0, 1)
    trx(1)
    mm1(0)
    copy_lkT(0)
    trllk(0)
    copy_xT(1, 0)
    copy_xT(1, 1)
    softmax(0)
    mm1(1)
    copy_lkT(1)
    trllk(1)
    mm2(0)
    softmax(1)
    epilogue(0)
    mm2(1)
    epilogue(1)
    ctx.close()


_NC = None


def _get_nc():
    global _NC
    if _NC is None:
        import concourse.bacc as bacc
        import concourse.tile as tile
        from concourse import mybir

        f32 = mybir.dt.float32
        nc = bacc.Bacc(
            "TRN2", target_bir_lowering=False, debug=False, num_devices=N_CORES
        )
        x_d = nc.dram_tensor("x", [B_LOC, T, D], f32, kind="ExternalInput").ap()
        mu_d = nc.dram_tensor("mu", [K, D], f32, kind="ExternalInput").ap()
        prec_d = nc.dram_tensor("prec", [K], f32, kind="ExternalInput").ap()
        out_d = nc.dram_tensor(
            "out", [B_LOC, K * D], f32, kind="ExternalOutput"
        ).ap()
        with tile.TileContext(nc) as tc:
            _emit(tc, x_d, mu_d, prec_d, out_d)
        nc.compile()
        _NC = nc
    return _NC


def kernel(x, mu, prec, **_ignored):
    from concourse.bass_utils import run_bass_kernel_spmd

    x = np.ascontiguousarray(np.asarray(x, dtype=np.float32))
    mu = np.ascontiguousarray(np.asarray(mu, dtype=np.float32))
    prec = np.ascontiguousarray(np.asarray(prec, dtype=np.float32))
    nc = _get_nc()
    in_maps = [
        {"x": x[c * B_LOC : (c + 1) * B_LOC], "mu": mu, "prec": prec}
        for c in range(N_CORES)
    ]
    res = run_bass_kernel_spmd(nc, in_maps, list(range(N_CORES)))
    return np.concatenate(
        [res.results[c]["out"] for c in range(N_CORES)], axis=0
    ).astype(np.float32)


# revision 16
# speedup vs baseline: 1.1559x; 1.0012x over previous
"""LDEPool1d Trainium2 Bass kernel (v4).

Reference computation (B=16, T=800, D=256, K=64):
    delta = x[:,:,None,:] - mu[None,None,:,:]          # (B,T,K,D)
    dist  = sum(delta*delta, -1)                       # (B,T,K)
    llk   = -(prec*prec) * dist
    r     = softmax(llk, axis=-1)                      # over K
    r     = r / (sum(r, axis=1) + 1e-9)                # over T
    pool  = einsum('btk,btkd->bkd', r, delta)          # (B,K,D)
    out   = pool.reshape(B, K*D)

Kernel algebra (per batch b; prec is constant so -p2*||x_t||^2 cancels in
the softmax over k):
    G[t,k] = sum_d x[t,d] * (2*p2[k]*mu[k,d])          (mm1, fp32 [t,k])
    llk    = G + nb[k],  nb = -p2*||mu_k||^2
    The softmax bias uses C_t = rowmax(G) + nbmax - 40 so that
    e = exp(G - rowmax(G) + (nb - nbmax + 40)) stays in [~e-46, e+40]:
    no overflow, and components with tiny weights keep full relative
    precision (needed: dead components with S ~ 1e-9 amplify relative r
    errors by ~1/(S+1e-9) in the T-normalization).
    Z_t = sum_k e, r = e / Z_t  (the +40/nbmax shift cancels in r)
    S_k = sum_t r  (ones-column in mm2), M2 = r^T @ x  (mm2, fp32r)
    out = M2 * Sr - mu * (S*Sr),  Sr = 1/(S+1e-9)

mm1 computes llkT[k,t] = mu_s @ x^T with mu stationary and xT moving at
N=400 so the fp32r fast path (1 cyc/row) applies; mu_s is split into a
bf16 high part + fp32 residual (two accumulating matmuls) so only x's
~10-bit fp32r truncation remains (llk errors are exponentiated, and
dead components with S ~ 1e-9 amplify relative r errors by ~1/(S+1e-9),
so mu-side sloppiness would cost ~1e-2).  The transpose-back of llkT to
[t,k] runs in exact fp32.  Sharding: data-parallel over B across 8
cores (2 batches/core), mu/prec replicated.  No collectives.
"""

import sys

if "/opt/trn_rl_repo" not in sys.path:
    sys.path.insert(0, "/opt/trn_rl_repo")

import numpy as np

B, T, D, K = 16, 800, 256, 64
N_CORES = 8
B_LOC = B // N_CORES  # batches per core
EPS = 1e-9
ONE_F32_BITS = 0x3F800000

# T-chunks of <=128 rows (SBUF partition dim)
CHUNKS = [(t0, min(128, T - t0)) for t0 in range(0, T, 128)]
NCH = len(CHUNKS)  # 7: 6 x 128 + 32
TP = NCH * 128  # xT padded with zero t-columns so mm1 writes full rows


def _bc(ap, n):
    """Append a stride-0 inner dim of size n to an AP (broadcast)."""
    import concourse.bass as bass

    return bass.AP(ap.tensor, ap.offset, ap.ap + [[0, n]])


def _bc_mid(ap, n):
    """Insert a stride-0 dim of size n before the last dim of an AP."""
    import concourse.bass as bass

    return bass.AP(ap.tensor, ap.offset, ap.ap[:-1] + [[0, n]] + [ap.ap[-1]])


def _emit(tc, x_d, mu_d, prec_d, out_d):
    from concourse import mybir
    from concourse.masks import make_identity
    from contextlib import ExitStack

    f32 = mybir.dt.float32
    f32r = mybir.dt.float32r
    u32 = mybir.dt.uint32
    nc = tc.nc
    AF = mybir.ActivationFunctionType
    OP = mybir.AluOpType

    def r32(ap):
        return ap.bitcast(f32r)

    ctx = ExitStack()
    const = ctx.enter_context(tc.tile_pool(name="const", bufs=1))
    xta = ctx.enter_context(tc.tile_pool(name="ps_xta", bufs=2, space="PSUM"))
    xtb = ctx.enter_context(tc.tile_pool(name="ps_xtb", bufs=2, space="PSUM"))
    lkt = ctx.enter_context(tc.tile_pool(name="ps_lkt", bufs=1, space="PSUM"))
    lkp = ctx.enter_context(tc.tile_pool(name="ps_llk", bufs=1, space="PSUM"))
    ppp = ctx.enter_context(tc.tile_pool(name="ps_pp", bufs=1, space="PSUM"))

    # ---------------- constants / per-batch SBUF tiles ----------------
    identity = const.tile([128, 128], f32r)  # for the fp32r x transposes
    identity_f = const.tile([64, 64], f32)  # for the fp32 setup transposes
    mu_nat = const.tile([K, D], f32)
    prec_sb = const.tile([K, 1], f32)
    p2 = const.tile([K, 1], f32)
    p22 = const.tile([K, 1], f32)
    mu_s = const.tile([K, D], f32)
    sq = const.tile([K, D], f32)
    musq = const.tile([K, 1], f32)
    nb = const.tile([K, 1], f32)
    muTh = const.tile([128, 2, K], f32r)  # fp32r-rounded high part
    muTl = const.tile([128, 2, K], f32r)  # residual
    nbs_row = const.tile([1, K], f32)  # nb - nbmax + 40, as a row
    nbmax = const.tile([1, 1], f32)
    nbsh = const.tile([1, 1], f32)
    ones_col = const.tile([1, 128], f32)
    nbs_rep = const.tile([128, K], f32)  # nbs_row replicated to all parts
    dum = const.tile([1, 1], f32)

    x_sb, xT_sb, lkT_sb, epre, e_sb, r_sb = [], [], [], [], [], []
    nm, nmnb, z, zinv, se, sr, c1, t1, t2, po = ([] for _ in range(10))
    for b in range(B_LOC):
        x_sb.append(const.tile([128, NCH, D + 2], f32r, tag=f"x{b}", name=f"x{b}"))
        xT_sb.append(const.tile([128, 2, T], f32r, tag=f"xT{b}", name=f"xT{b}"))
        lkT_sb.append(const.tile([64, TP], f32, tag=f"lkT{b}", name=f"lkT{b}"))
        epre.append(const.tile([128, NCH, K], f32, tag=f"ep{b}", name=f"ep{b}"))
        e_sb.append(const.tile([128, NCH, K], f32, tag=f"e{b}", name=f"e{b}"))
        r_sb.append(const.tile([128, NCH, K], f32r, tag=f"r{b}", name=f"r{b}"))
        nm.append(const.tile([128, NCH], f32, tag=f"nm{b}", name=f"nm{b}"))
        nmnb.append(const.tile([128, NCH, K], f32, tag=f"nn{b}", name=f"nn{b}"))
        z.append(const.tile([128, NCH], f32, tag=f"z{b}", name=f"z{b}"))
        zinv.append(const.tile([128, NCH], f32, tag=f"zi{b}", name=f"zi{b}"))
        se.append(const.tile([K, 1], f32, tag=f"se{b}", name=f"se{b}"))
        sr.append(const.tile([K, 1], f32, tag=f"sr{b}", name=f"sr{b}"))
        c1.append(const.tile([K, 1], f32, tag=f"c1{b}", name=f"c1{b}"))
        t1.append(const.tile([K, D], f32, tag=f"t1{b}", name=f"t1{b}"))
        t2.append(const.tile([K, D], f32, tag=f"t2{b}", name=f"t2{b}"))
        po.append(const.tile([K, D], f32, tag=f"po{b}", name=f"po{b}"))

    # ---------------- setup ----------------
    # Prefetch the exp table set on ACT before anything else needs ACT.
    nc.gpsimd.memset(dum, 0.0)
    nc.scalar.activation(dum, dum, AF.Exp)

    # mu/prec first so the mu math can start immediately; x in 3 chunks
    # per batch (big DMAs, but the first lands early enough for trx).
    def dma_x(b, part):
        if part == 0:
            nc.sync.dma_start(
                out=x_sb[b][:, 0:3, 0:D],
                in_=x_d[b, 0:384, :].rearrange("(c p) d -> p c d", p=128).bitcast(f32r),
            )
        elif part == 1:
            nc.sync.dma_start(
                out=x_sb[b][:, 3:6, 0:D],
                in_=x_d[b, 384:768, :].rearrange("(c p) d -> p c d", p=128).bitcast(f32r),
            )
        else:
            nc.sync.dma_start(
                out=x_sb[b][0:32, 6, 0:D], in_=x_d[b, 768:800, :].bitcast(f32r)
            )

    dma_x(0, 0)
    nc.sync.dma_start(out=mu_nat, in_=mu_d)
    nc.sync.dma_start(out=prec_sb, in_=prec_d)
    dma_x(0, 1)
    dma_x(0, 2)
    dma_x(1, 0)
    dma_x(1, 1)
    dma_x(1, 2)

    # Identities. The f32r one is built in place: uint32 memset for the
    # zeros (no f32r memset encoding exists) + affine_select for the diag.
    nc.gpsimd.memset(identity.bitcast(u32), 0)
    make_identity(nc, identity, nomemset=True)
    make_identity(nc, identity_f)
    nc.gpsimd.memset(ones_col, 1.0)
    for b in range(B_LOC):
        nc.gpsimd.memset(lkT_sb[b][:, T:TP], 0.0)
        # ones col for S_k (+ zero col so the fp32r moving dim is even)
        nc.gpsimd.memset(x_sb[b][:, :, D : D + 1].bitcast(u32), ONE_F32_BITS)
        nc.gpsimd.memset(x_sb[b][:, :, D + 1 : D + 2].bitcast(u32), 0)

    # mu math: p2 = prec^2; mu_s = 2*p2*mu; nb = -p2*||mu||^2
    nc.vector.tensor_mul(p2, prec_sb, prec_sb)
    nc.vector.tensor_scalar_mul(p22, p2, 2.0)
    nc.scalar.activation(mu_s, mu_nat, AF.Copy, scale=p22)
    nc.vector.tensor_mul(sq, mu_nat, mu_nat)
    nc.vector.tensor_reduce(
        out=musq, in_=sq, axis=mybir.AxisListType.X, op=OP.add
    )
    nc.vector.tensor_mul(nb, p2, musq)
    nc.vector.tensor_scalar_mul(nb, nb, -1.0)

    def setup_mu_transposes():
        # Transpose mu_s halves and nb via PE (fp32), staged through one
        # slot of the llk psum pool; split mu into an fp32r-rounded high
        # part + residual straight from PSUM.
        wrp = lkp.tile([128, NCH, K], f32, tag="llk")
        nc.tensor.transpose(wrp[:, 0, :], mu_s[:, 0:128], identity_f)
        nc.tensor.transpose(wrp[:, 1, :], mu_s[:, 128:256], identity_f)
        nc.tensor.transpose(wrp[0:1, 4, :], nb[:, 0:1], identity_f)
        nc.scalar.copy(muTh, wrp[:, 0:2, :])  # rounds to f32r
        nc.vector.tensor_tensor(
            out=muTl, in0=wrp[:, 0:2, :], in1=muTh, op=OP.subtract
        )
        # nbs_row = nb - nbmax + 40 (exponent offset; cancels in r)
        nc.vector.tensor_reduce(
            out=nbmax, in_=wrp[0:1, 4, :], axis=mybir.AxisListType.X,
            op=OP.max, negate=True,
        )
        nc.vector.tensor_scalar_add(nbsh, nbmax, 40.0)
        nc.vector.tensor_scalar(
            out=nbs_row, in0=wrp[0:1, 4, :], scalar1=nbsh, scalar2=None,
            op0=OP.add,
        )
        # Replicate nbs_row across all 128 partitions (rank-1 matmul).
        nc.tensor.matmul(wrp[:, 5, :], lhsT=ones_col, rhs=nbs_row)
        nc.scalar.copy(nbs_rep, wrp[:, 5, :])

    # ---------------- per-batch pipeline stages ----------------
    state = {}

    def trx(b):
        """Transpose x into xT (PSUM); fp32r (1.5 cyc/row)."""
        st = state.setdefault(b, {})
        st["xtps"] = []
        for h in range(2):
            pa = xta.tile([128, 512], f32, tag="xta")
            pb = xtb.tile([128, 288], f32, tag="xtb")
            for c, (t0, tcn) in enumerate(CHUNKS):
                dst = (
                    pa[:, t0 : t0 + tcn]
                    if t0 + tcn <= 512
                    else pb[:, t0 - 512 : t0 - 512 + tcn]
                )
                nc.tensor.matmul(
                    r32(dst),
                    lhsT=x_sb[b][0:tcn, c, h * 128 : (h + 1) * 128],
                    rhs=identity[0:tcn, 0:tcn],
                    is_transpose=True,
                )
            st["xtps"].append((pa, pb))

    def copy_xT(b, h):
        pa, pb = state[b]["xtps"][h]
        eng = nc.scalar if h == 0 else nc.vector
        if h == 0:
            eng.copy(xT_sb[b][:, h, 0:512], pa)
            eng.copy(xT_sb[b][:, h, 512:T], pb)
        else:
            eng.tensor_copy(xT_sb[b][:, h, 0:512], pa)
            eng.tensor_copy(xT_sb[b][:, h, 512:T], pb)

    def mm1(b):
        """llkT[k,t] = sum_d mu_s[k,d] x[t,d]; fp32r, mu stationary N=400."""
        pt = lkt.tile([64, 2, 512], f32, tag="lkT")
        for tc in range(2):
            for j in range(4):  # (h0,hi) (h0,lo) (h1,hi) (h1,lo)
                h, lo = j // 2, j % 2
                nc.tensor.matmul(
                    pt[:, tc, 0:400],
                    lhsT=(muTl if lo else muTh)[:, h, :],
                    rhs=xT_sb[b][:, h, tc * 400 : (tc + 1) * 400],
                    start=(j == 0),
                    stop=(j == 3),
                )
        state[b]["lkTps"] = pt

    def copy_lkT(b):
        pt = state[b]["lkTps"]
        nc.scalar.copy(lkT_sb[b][:, 0:400], pt[:, 0, 0:400])
        nc.scalar.copy(lkT_sb[b][:, 400:T], pt[:, 1, 0:400])

    def trllk(b):
        """llk[t,k] = llkT^T via exact fp32 PE transposes."""
        pl = lkp.tile([128, NCH, K], f32, tag="llk")
        for c in range(NCH):
            nc.tensor.transpose(
                pl[:, c, :],
                lkT_sb[b][:, c * 128 : (c + 1) * 128],
                identity_f,
            )
        state[b]["llkps"] = pl

    def softmax(b):
        pl = state[b]["llkps"]
        # Three back-to-back DVE ops (no cross-engine hops): shift by the
        # static per-k offset, take the true row max, subtract it.
        nc.vector.tensor_tensor(
            out=nmnb[b], in0=pl, in1=_bc_mid(nbs_rep, NCH), op=OP.add
        )
        nc.vector.tensor_reduce(
            out=nm[b], in_=nmnb[b], axis=mybir.AxisListType.X, op=OP.max,
            negate=True,
        )
        nc.vector.tensor_tensor(
            out=epre[b], in0=nmnb[b], in1=_bc(nm[b], K), op=OP.add
        )
        nc.scalar.activation(e_sb[b], epre[b], AF.Exp)
        nc.vector.tensor_reduce(
            out=z[b], in_=e_sb[b], axis=mybir.AxisListType.X, op=OP.add
        )
        nc.vector.reciprocal(zinv[b], z[b])
        nc.gpsimd.tensor_tensor(
            out=r_sb[b], in0=e_sb[b], in1=_bc(zinv[b], K), op=OP.mult
        )

    def mm2(b):
        pp = ppp.tile([K, D + 2], f32, tag="pp")
        for c, (t0, tcn) in enumerate(CHUNKS):
            nc.tensor.matmul(
                pp,
                lhsT=r_sb[b][0:tcn, c, :],
                rhs=x_sb[b][0:tcn, c, 0 : D + 2],
                start=(c == 0),
                stop=(c == NCH - 1),
            )
        state[b]["pp"] = pp

    def epilogue(b):
        pp = state[b]["pp"]
        nc.vector.tensor_scalar_add(se[b], pp[:, D : D + 1], EPS)
        nc.vector.reciprocal(sr[b], se[b])
        nc.vector.tensor_mul(c1[b], pp[:, D : D + 1], sr[b])
        nc.scalar.activation(t2[b], pp[:, 0:D], AF.Copy, scale=sr[b])
        nc.scalar.activation(t1[b], mu_nat, AF.Copy, scale=c1[b])
        nc.gpsimd.tensor_tensor(out=po[b], in0=t2[b], in1=t1[b], op=OP.subtract)
        nc.sync.dma_start(
            out=out_d[b, :].rearrange("(k d) -> k d", k=K), in_=po[b]
        )

    # Interleave the two batches to keep PE busy during softmax/copies.
    setup_mu_transposes()
    trx(0)
    copy_xT(0, 0)
    copy_xT(0, 1)
    trx(1)
    mm1(0)
    copy_lkT(0)
    trllk(0)
    copy_xT(1, 0)
    copy_xT(1, 1)
    softmax(0)
    mm1(1)
    copy_lkT(1)
    trllk(1)
    mm2(0)
    softmax(1)
    epilogue(0)
    mm2(1)
    epilogue(1)
    ctx.close()


_NC = None


def _get_nc():
    global _NC
    if _NC is None:
        import concourse.bacc as bacc
        import concourse.tile as tile
        from concourse import mybir

        f32 = mybir.dt.float32
        nc = bacc.Bacc(
            "TRN2", target_bir_lowering=False, debug=False, num_devices=N_CORES
        )
        x_d = nc.dram_tensor("x", [B_LOC, T, D], f32, kind="ExternalInput").ap()
        mu_d = nc.dram_tensor("mu", [K, D], f32, kind="ExternalInput").ap()
        prec_d = nc.dram_tensor("prec", [K], f32, kind="ExternalInput").ap()
        out_d = nc.dram_tensor(
            "out", [B_LOC, K * D], f32, kind="ExternalOutput"
        ).ap()
        with tile.TileContext(nc) as tc:
            _emit(tc, x_d, mu_d, prec_d, out_d)
        nc.compile()
        _NC = nc
    return _NC


def kernel(x, mu, prec, **_ignored):
    from concourse.bass_utils import run_bass_kernel_spmd

    x = np.ascontiguousarray(np.asarray(x, dtype=np.float32))
    mu = np.ascontiguousarray(np.asarray(mu, dtype=np.float32))
    prec = np.ascontiguousarray(np.asarray(prec, dtype=np.float32))
    nc = _get_nc()
    in_maps = [
        {"x": x[c * B_LOC : (c + 1) * B_LOC], "mu": mu, "prec": prec}
        for c in range(N_CORES)
    ]
    res = run_bass_kernel_spmd(nc, in_maps, list(range(N_CORES)))
    return np.concatenate(
        [res.results[c]["out"] for c in range(N_CORES)], axis=0
    ).astype(np.float32)
